# revision 8
# baseline (speedup 1.0000x reference)
"""GroupedQueryAttention on 8 Trainium2 NeuronCores.

Sharding: core c = 4*b + r handles batch b (of 2) and token chunk r (512
of 2048 tokens) for Q/attention/o_proj over ALL 16 heads. K/V projections
are sharded by KV group: core r computes group g=r's K/V for all T, then
one small (1 MB in / 4 MB out) AllGather across each batch's 4 cores makes
every core independent for the rest of the kernel -- no output collective.

AllGather payload kv_loc [256, 2048]: rows 0:128 K^T [hd, T]; rows
128:256 V in per-partition-major order (row 128+p, cols s*128+c holds
V[s*128+p, c]) so the consumer-side unpack is one fat [128, 2048] DMA
per group instead of 64 short-line DMAs.

Per-group token order is core-local (own chunk first): softmax + A@V are
permutation-invariant over keys as long as K and V share the order.

DMA queues: Sync carries the x^T stream, the wq stream, unpacks and
output writes; Activation carries x_own, wk/wv/biases, K/V AllGather
input writes, and the wo stream. This keeps the latency-critical
AllGather inputs and the Q-proj weight stream on independent queues.

PSUM: tag "big" = 2 bufs x [128, 1024] fp32 (scores for TWO key blocks
per buffer -> one batched exp instruction, halving Activation-engine
instruction+semaphore overhead, which paced v2's attention phase); tag
"opk" = 4 bufs x [128, 129] fp32 A@V accumulators, whose banks also host
the transient transpose outputs.

All matmuls run in fp16 (1 PE cycle/row) with fp32 PSUM accumulation.
Layouts avoid transposing the big P matrix:
  - projections produce Q^T/K^T/V^T directly (lhsT=W block, rhs=x^T block)
  - scores are computed as S^T = (K^T).T @ Q^T
  - exp(S^T) = P^T feeds A@V as lhsT directly
  - V carries an extra ones-column so the softmax denominator falls out
    of the A@V matmul for free
  - o_proj bias is added via an identity-matmul of a broadcast bias row
"""

import math
import sys

import numpy as np

sys.path.insert(0, "/opt/trn_rl_repo")

B = 2
T = 2048
D = 2048
HEADS = 16
GROUPS = 4
HD = 128  # head dim
M = HEADS // GROUPS  # heads per group = 4
SCALE = 1.0 / math.sqrt(HD)
N_CORES = 8
TCH = 512  # token chunk per core
NTCH = T // TCH  # 4
NSB = T // 128  # 16 key blocks
NKS = D // 128  # 16 contraction steps for projections
NQ = NKS // 4  # 4 quad blocks for the x stream
NNB = D // TCH  # 4 o_proj output column blocks

_COMPILED = {}


def _build():
    import concourse.bass as bass
    import concourse.mybir as mybir
    import concourse.tile as tile
    from concourse import bacc
    from concourse.masks import make_identity

    f16 = mybir.dt.float16
    f32 = mybir.dt.float32
    Exp = mybir.ActivationFunctionType.Exp

    nc = bacc.Bacc("TRN2", target_bir_lowering=False, num_devices=N_CORES)

    # x^T as (slot, quad) row-blocks of [128, 2048]; slot 0 = own chunk
    xcb_d = nc.declare_dram_parameter("xcb", [NTCH * NQ * 128, 4 * TCH], f16,
                                      isOutput=False)
    wq_d = nc.declare_dram_parameter("wq", [128, HEADS * NKS * 128], f16,
                                     isOutput=False)
    wk_d = nc.declare_dram_parameter("wk", [128, NKS * 128], f16, isOutput=False)
    wv_d = nc.declare_dram_parameter("wv", [128, NKS * 128], f16, isOutput=False)
    wo_d = nc.declare_dram_parameter("wo", [128, NNB * HEADS * TCH], f16,
                                     isOutput=False)
    bqs_d = nc.declare_dram_parameter("bqs", [128, HEADS], f32, isOutput=False)
    bks_d = nc.declare_dram_parameter("bks", [128, 1], f32, isOutput=False)
    bvs_d = nc.declare_dram_parameter("bvs", [128, 1], f32, isOutput=False)
    bob_d = nc.declare_dram_parameter("bob", [128, D], f16, isOutput=False)
    out_d = nc.declare_dram_parameter("out", [TCH, D], f32, isOutput=True)

    groups = [[0, 1, 2, 3], [4, 5, 6, 7]]

    with tile.TileContext(nc) as tc:
        with (
            tc.tile_pool(name="const", bufs=1) as const,
            tc.tile_pool(name="work", bufs=2) as work,
            tc.tile_pool(name="psum", bufs=1, space="PSUM") as psum,
            tc.tile_pool(name="dram", bufs=1, space="DRAM") as dram,
        ):
            ident = const.tile([128, 128], f16)
            make_identity(nc, ident)
            bqs = const.tile([128, HEADS], f32)
            bks = const.tile([128, 1], f32)
            bvs = const.tile([128, 1], f32)
            bob = const.tile([128, D], f16)

            wk_sb = const.tile([128, NKS, 128], f16)
            wv_sb = const.tile([128, NKS, 128], f16)
            x_own = const.tile([128, NQ, 4 * TCH], f16)
            # act-queue loads: needed by phase 1 / early phase 2
            nc.scalar.dma_start(wk_sb[:], wk_d[:])
            nc.scalar.dma_start(wv_sb[:], wv_d[:])
            nc.scalar.dma_start(bks[:], bks_d[:])
            nc.scalar.dma_start(bvs[:], bvs_d[:])
            nc.scalar.dma_start(bqs[:], bqs_d[:])
            for q in range(NQ):
                nc.scalar.dma_start(x_own[:, q, :], xcb_d[q * 128 : (q + 1) * 128, :])

            kt = const.tile([128, GROUPS, T], f16)  # gathered K^T
            v_sb = const.tile([128, GROUPS, NSB, 132], f16)  # gathered V + ones
            qt = const.tile([128, HEADS, TCH], f16)  # own-chunk Q^T
            at = const.tile([128, HEADS, TCH], f16)  # own-chunk A^T
            nc.vector.memset(v_sb[:, :, :, 128:129], 1.0)

            # AllGather payload: rows 0:128 K^T, rows 128:256 V p-major
            kv_loc = dram.tile([256, T], f16, tag="kvl", name="kv_loc")
            kv_g = dram.tile([GROUPS * 256, T], f16, tag="kvg", name="kv_g")

            # wq stream: first two on the act queue up front, rest on sync
            NWQB = 4
            wq_tiles = {}

            def issue_wq(h, eng=None):
                wqh = work.tile([128, NKS * 128], f16, tag="wq", bufs=NWQB,
                                name="wqh", uniquify=True)
                (eng or nc.sync).dma_start(
                    wqh[:], wq_d[:, h * NKS * 128 : (h + 1) * NKS * 128])
                wq_tiles[h] = wqh

            issue_wq(0, nc.scalar)
            issue_wq(1, nc.scalar)

            # ---- phase 1: K/V projection for own group, all T ----
            for tc2 in [1, 2, 3, 0]:
                kacc = psum.tile([128, TCH], f32, tag="big", bufs=2, name="kacc")
                vacc = psum.tile([128, TCH], f32, tag="big", bufs=2, name="vacc")
                for q in range(NQ):
                    if tc2 == 0:
                        x4 = x_own[:, q, :]
                    else:
                        x4t = work.tile([128, 4 * TCH], f16, tag="xs", bufs=3,
                                        name="x4t")
                        nc.sync.dma_start(
                            x4t[:],
                            xcb_d[(tc2 * NQ + q) * 128 : (tc2 * NQ + q + 1) * 128, :],
                        )
                        x4 = x4t[:]
                    for k2 in range(4):
                        ks = q * 4 + k2
                        xb = x4[:, k2 * TCH : (k2 + 1) * TCH]
                        nc.tensor.matmul(
                            kacc[:], wk_sb[:, ks, :], xb,
                            start=(ks == 0), stop=(ks == NKS - 1),
                        )
                        nc.tensor.matmul(
                            vacc[:], wv_sb[:, ks, :], xb,
                            start=(ks == 0), stop=(ks == NKS - 1),
                        )
                ktc = work.tile([128, TCH], f16, tag="ktc", bufs=2, name="ktc")
                nc.vector.tensor_scalar_add(ktc[:], kacc[:], bks[:, 0:1])
                nc.scalar.dma_start(kv_loc[0:128, tc2 * TCH : (tc2 + 1) * TCH], ktc[:])
                vtc = work.tile([128, TCH], f16, tag="vtc", bufs=2, name="vtc")
                nc.vector.tensor_scalar_add(vtc[:], vacc[:], bvs[:, 0:1])
                for sb in range(4):
                    sbg = tc2 * 4 + sb
                    tp = psum.tile([128, 128], f16, tag="opk", bufs=4, name="tp")
                    nc.tensor.transpose(tp[:], vtc[:, sb * 128 : (sb + 1) * 128],
                                        ident[:])
                    vn = work.tile([128, 128], f16, tag="vn", bufs=3, name="vn")
                    nc.vector.tensor_copy(vn[:], tp[:])
                    # V block, p-major: row 128+p, cols sbg*128..+128
                    nc.scalar.dma_start(
                        kv_loc[128:256, sbg * 128 : (sbg + 1) * 128], vn[:]
                    )

            nc.gpsimd.collective_compute(
                "AllGather", mybir.AluOpType.bypass, replica_groups=groups,
                ins=[kv_loc[:]], outs=[kv_g[:]],
            )
            issue_wq(2)
            issue_wq(3)

            # unpack gathered K^T / V into SBUF on the act queue (idle here;
            # triggers wait on the AllGather semaphore, then run immediately)
            for g in range(GROUPS):
                nc.scalar.dma_start(kt[:, g, :], kv_g[g * 256 : g * 256 + 128, :])
                nc.scalar.dma_start(
                    v_sb[:, g, :, 0:128], kv_g[g * 256 + 128 : (g + 1) * 256, :]
                )

            # ---- phase 2: Q projection (own chunk, all heads); overlaps AG ----
            for h in range(HEADS):
                wqh = wq_tiles[h]
                qacc = psum.tile([128, TCH], f32, tag="big", bufs=2, name="qacc")
                for ks in range(NKS):
                    nc.tensor.matmul(
                        qacc[:], wqh[:, ks * 128 : (ks + 1) * 128],
                        x_own[:, ks // 4, (ks % 4) * TCH : (ks % 4 + 1) * TCH],
                        start=(ks == 0), stop=(ks == NKS - 1),
                    )
                nc.vector.tensor_scalar(
                    qt[:, h, :], qacc[:], SCALE, bqs[:, h : h + 1],
                    op0=mybir.AluOpType.mult, op1=mybir.AluOpType.add,
                )
                if h + NWQB < HEADS:
                    issue_wq(h + NWQB)

            # ---- phase 3: attention for own chunk, all heads ----
            # Normalize/transpose of head h-1 is issued after head h's first
            # score pair so the PE never stalls on the DVE normalize chain.
            pending = []

            def normalize(h, opks):
                for tb in range(4):
                    opk = opks[tb]
                    rcp = work.tile([128, 1], f32, tag="rcp", bufs=2, name="rcp")
                    nc.vector.reciprocal(rcp[:], opk[:, 128:129])
                    o_sb = work.tile([128, 128], f16, tag="osb", bufs=2, name="osb")
                    nc.vector.tensor_scalar_mul(o_sb[:], opk[:, 0:128], rcp[:])
                    tp = psum.tile([128, 128], f16, tag="big", bufs=2, name="tpo")
                    nc.tensor.transpose(tp[:], o_sb[:], ident[:])
                    nc.vector.tensor_copy(at[:, h, tb * 128 : (tb + 1) * 128],
                                          tp[:])

            for g in range(GROUPS):
                for hh in range(M):
                    h = g * M + hh
                    opks = [
                        psum.tile([128, 129], f32, tag="opk", bufs=4, name=f"opk{i}")
                        for i in range(4)
                    ]
                    for sp in range(NSB // 2):
                        sps2 = psum.tile([128, 2 * TCH], f32, tag="big", bufs=2,
                                         name="sps2")
                        for j in range(2):
                            s = sp * 2 + j
                            nc.tensor.matmul(
                                sps2[:, j * TCH : (j + 1) * TCH],
                                kt[:, g, s * 128 : (s + 1) * 128], qt[:, h, :],
                                start=True, stop=True,
                            )
                        if sp == 0 and pending:
                            normalize(*pending.pop())
                        p2 = work.tile([128, 2 * TCH], f16, tag="p", bufs=4,
                                       name="p2")
                        nc.scalar.activation(p2[:], sps2[:], Exp)
                        for j in range(2):
                            s = sp * 2 + j
                            for tb in range(4):
                                nc.tensor.matmul(
                                    opks[tb][:, 0:129],
                                    p2[:, j * TCH + tb * 128 : j * TCH + (tb + 1) * 128],
                                    v_sb[:, g, s, 0:129],
                                    start=(s == 0), stop=(s == NSB - 1),
                                )
                    pending.append((h, opks))
            normalize(*pending.pop())

            # ---- phase 4: o_proj for own chunk, full D ----
            nc.scalar.dma_start(bob[:], bob_d[:])
            for nb in range(NNB):
                wob = work.tile([128, HEADS * TCH], f16, tag="wo", bufs=2, name="wob")
                nc.scalar.dma_start(
                    wob[:], wo_d[:, nb * HEADS * TCH : (nb + 1) * HEADS * TCH]
                )
                for tb in range(4):
                    pp = psum.tile([128, TCH], f32, tag="big", bufs=2, name="pp")
                    # bias row via identity matmul: out[m, n] += bob[m, n]
                    nc.tensor.matmul(
                        pp[:], ident[:], bob[:, nb * TCH : (nb + 1) * TCH],
                        start=True, stop=False,
                    )
                    for h in range(HEADS):
                        nc.tensor.matmul(
                            pp[:],
                            at[:, h, tb * 128 : (tb + 1) * 128],
                            wob[:, h * TCH : (h + 1) * TCH],
                            start=False, stop=(h == HEADS - 1),
                        )
                    ob = work.tile([128, TCH], f32, tag="ob", bufs=3, name="ob")
                    nc.vector.tensor_copy(ob[:], pp[:])
                    nc.sync.dma_start(
                        out_d[tb * 128 : (tb + 1) * 128, nb * TCH : (nb + 1) * TCH],
                        ob[:],
                    )

    nc.compile()
    return nc


def _get_nc():
    if "nc" not in _COMPILED:
        _COMPILED["nc"] = _build()
    return _COMPILED["nc"]


def kernel(x, Wq, bq, Wk, bk, Wv, bv, Wo, bo):
    from concourse.bass_utils import run_bass_kernel_spmd

    x = np.asarray(x, np.float32)
    Wq = np.asarray(Wq, np.float32)
    Wk = np.asarray(Wk, np.float32)
    Wv = np.asarray(Wv, np.float32)
    Wo = np.asarray(Wo, np.float32)
    bq = np.asarray(bq, np.float32)
    bk = np.asarray(bk, np.float32)
    bv = np.asarray(bv, np.float32)
    bo = np.asarray(bo, np.float32)

    nc = _get_nc()

    # shared across cores
    wq_h = np.ascontiguousarray(
        Wq.reshape(NKS, 128, HEADS, 128).transpose(1, 2, 0, 3).reshape(128, -1)
    ).astype(np.float16)
    wo_h = np.ascontiguousarray(
        Wo.reshape(HEADS, 128, NNB, TCH).transpose(1, 2, 0, 3).reshape(128, -1)
    ).astype(np.float16)
    bqs_h = np.ascontiguousarray((bq * SCALE).reshape(HEADS, 128).T)
    bob_h = np.ascontiguousarray(np.broadcast_to(bo.astype(np.float16), (128, D)))
    # x^T per batch, pre-blocked into (chunk, quad) [128, 2048] row-blocks
    xq16 = []
    for b in range(B):
        xTb = x[b].T.astype(np.float16)  # [D, T]
        blocks = xTb.reshape(NKS, 128, NTCH, TCH).transpose(2, 0, 1, 3)
        # [chunk, ks, 128, TCH] -> quads: [chunk, quad, 128, 4*TCH]
        blocks = blocks.reshape(NTCH, NQ, 4, 128, TCH).transpose(0, 1, 3, 2, 4)
        xq16.append(np.ascontiguousarray(blocks.reshape(NTCH, NQ * 128, 4 * TCH)))

    wk_g, wv_g, bks_g, bvs_g = [], [], [], []
    for g in range(GROUPS):
        wk_g.append(
            np.ascontiguousarray(
                Wk[:, g * HD : (g + 1) * HD].reshape(NKS, 128, HD)
                .transpose(1, 0, 2).reshape(128, -1)
            ).astype(np.float16)
        )
        wv_g.append(
            np.ascontiguousarray(
                Wv[:, g * HD : (g + 1) * HD].reshape(NKS, 128, HD)
                .transpose(1, 0, 2).reshape(128, -1)
            ).astype(np.float16)
        )
        bks_g.append(np.ascontiguousarray(bk[g * HD : (g + 1) * HD].reshape(1, HD).T))
        bvs_g.append(np.ascontiguousarray(bv[g * HD : (g + 1) * HD].reshape(1, HD).T))

    in_maps = []
    for c in range(N_CORES):
        b, r = c // 4, c % 4
        order = [r] + [i for i in range(NTCH) if i != r]
        xcb = np.concatenate([xq16[b][s] for s in order], axis=0)
        in_maps.append(
            {
                "xcb": np.ascontiguousarray(xcb),
                "wq": wq_h,
                "wk": wk_g[r],
                "wv": wv_g[r],
                "wo": wo_h,
                "bqs": bqs_h,
                "bks": bks_g[r],
                "bvs": bvs_g[r],
                "bob": bob_h,
            }
        )

    res = run_bass_kernel_spmd(nc, in_maps, list(range(N_CORES)))
    _COMPILED["last_res"] = res

    out = np.empty((B, T, D), np.float32)
    for b in range(B):
        for r in range(NTCH):
            out[b, r * TCH : (r + 1) * TCH, :] = res.results[4 * b + r]["out"]
    return out
